# revision 14
# baseline (speedup 1.0000x reference)
"""HNLoRALinear Trainium2 kernel (bf16, packed-LoRA edition).

out[b,s,o] = x[b] @ W^T + bias + SCALE * (x[b] @ A[b]) @ B[b]

Sharding: 8 cores = 4 batches x 2 sequence-halves. Each core computes
its [1024 tokens, 4096 outs] output block, TRANSPOSED on device
(outs on PSUM partitions, tokens as the moving dim):
  - stationary operand: [128, 128] W^T chunk (bf16 -> fast weight load,
    fully hidden under the 512-column moving stream),
  - moving operand: 512-token slice of the SBUF-resident x^T (bf16),
  - consecutive k-chunk matmuls accumulate in fp32 PSUM.

All device data is bf16 (inputs rounded on host, output written bf16
and upcast on host): halves DMA traffic vs f32, enables the FWL
weight-load path, ~2.5e-3 rel err (gate is 2e-2).

Startup: ~10 dummy warm-up matmuls (HAM un-throttle) while the first
tiles stream in; all input DMAs ride ONE HWDGE ring (nc.sync) in
need-order (at, xg0, w0, xg1, w1, xg2, ..., xg15, w2, w3, bt) -- with
two rings the SDMA engines drain whole per-engine shares of one ring
before returning to the other, which stalled xg0's completion ~10us.
The first two o-strips' partial-k matmuls interleave with the x tile
arrivals so the PE has work for the whole x-load window.

LoRA work is packed into PE 32x32 sub-tiles:
  - low = (x @ A)^T: 4-way COLUMN tiling -- slots j=0..3 at
    tile_position (0, 32j) compute (t0, t1, t0, t1) concurrently
    (t-halves duplicated so each 32-row block of `low4` gets its values
    via a partition-ALIGNED PSUM->SBUF copy; no cross-partition moves).
  - the bias+lora stop matmul (K=17: [SCALE*B ; bias] @ [low ; ones]):
    4-way ROW tiling across a pair of o-strips -- slots i=2s+t at
    tile_position (32i, 0) read bt4/low4 rows 32i..32i+16.
This turns 128 full-cost matmuls into ~48 packed spans.
"""
import numpy as np
import ml_dtypes

import concourse.bass as bass  # noqa: F401  (bass must import before tile)
import concourse.mybir as mybir
import concourse.tile as tile
from concourse import bacc
from concourse.bass_utils import run_bass_kernel_spmd

# Problem shapes (hardcoded per contract).
B, S, D_IN, D_OUT, R = 4, 2048, 4096, 4096, 16
XG = 4                 # x DMA groups (separate tiles so deps are per-group)
SCALE = 32.0 / 16.0
SH = S // 2            # tokens per core
P = 128
KC = D_IN // P         # 32 contraction chunks
O_CHUNKS = D_OUT // P  # 32 output-feature chunks (PSUM partition dim)
TN = 512               # moving-dim token group width
TGROUPS = SH // TN     # 2
KG = KC // XG          # k-chunks per x group
RA = R + 1             # augmented rank (lora + bias row)
N_START = 2            # o-strips whose partial-k matmuls interleave with x load
N_WARM = 12            # dummy warm-up matmuls (HAM un-throttle during DMA wait)

BF16 = ml_dtypes.bfloat16

_cached_nc = None


def _build():
    bf16 = mybir.dt.bfloat16
    f32 = mybir.dt.float32
    nc = bacc.Bacc(
        "TRN2", target_bir_lowering=False, debug=False, enable_asserts=False
    )
    xt = nc.dram_tensor("xt", [XG, P, KG * SH], bf16, kind="ExternalInput")
    wt = nc.dram_tensor("wt", [O_CHUNKS, P, KC * P], bf16, kind="ExternalInput")
    apk = nc.dram_tensor("apack", [P, KC * R], bf16, kind="ExternalInput")
    bga = nc.dram_tensor("baug", [P, D_OUT], bf16, kind="ExternalInput")
    ot_d = nc.dram_tensor("ot", [D_OUT, SH], bf16, kind="ExternalOutput")

    with tile.TileContext(nc) as tc:
        with (
            tc.tile_pool(name="xp", bufs=1) as xp,
            tc.tile_pool(name="wp", bufs=5) as wp,
            tc.tile_pool(name="cp", bufs=1) as cp,
            tc.tile_pool(name="op", bufs=3) as op,
            tc.tile_pool(name="pp", bufs=8, space="PSUM") as pp,
        ):
            # HAM warm-up: the PE clock sits at 1.2 GHz until it has seen
            # ~3.4us of sustained matmul activity. Burn that window on dummy
            # matmuls over a memset tile (no DMA dependency) while the first
            # x/W tiles stream in, so the real matmuls run at 2.4 GHz.
            wu_sb = cp.tile([P, TN], bf16, name="wusb")
            nc.vector.memset(wu_sb[:], 0.0)
            wu_ps = pp.tile([P, TN], f32, name="ps")
            for i in range(N_WARM):
                nc.tensor.matmul(
                    wu_ps[:],
                    wu_sb[:, 0:P],
                    wu_sb[:],
                    start=(i == 0),
                    stop=(i == N_WARM - 1),
                )

            # Single-ring input DMA schedule, in PE-need order.
            at = cp.tile([P, KC * R], bf16, name="at")
            nc.sync.dma_start(out=at[:], in_=apk.ap())

            xgs = [
                xp.tile([P, KG * SH], bf16, name=f"xg{g}", tag=f"xg{g}")
                for g in range(XG)
            ]
            w_strips = {}

            def load_w_strip(o):
                # One fully-contiguous 2D DMA per strip (host pre-packs W
                # as [o_chunk, partition, k*128+c]).
                wk = wp.tile([P, KC * P], bf16, name="wk")
                nc.scalar.dma_start(out=wk[:], in_=wt.ap()[o])
                w_strips[o] = wk

            # Strips 0/1 stream in k-halves: with KG=8, half h of a strip
            # covers k-groups the startup block reaches after x-group 2h,
            # so it never waits on a whole-strip load.
            HW_ = KC * P // 2
            for o in range(N_START):
                w_strips[o] = wp.tile([P, KC * P], bf16, name="wk")

            def load_w_half(o, h):
                nc.scalar.dma_start(
                    out=w_strips[o][:, h * HW_ : (h + 1) * HW_],
                    in_=wt.ap()[o][:, h * HW_ : (h + 1) * HW_],
                )

            # Two HWDGE rings in parallel: x (+ outputs) on the SP ring,
            # all W/bias traffic on the Activation ring. Few, big DMAs:
            # per-DMA ring/completion overhead (~1.5us) is what starves
            # the startup, not bytes.
            # xg0 in quarters / xg1 in halves: the PE comes off warm-up at
            # ~12us and the first 512KB piece lands ~3us sooner than a
            # whole 2MB tile would, keeping HAM warm straight into real
            # work. Later tiles are whole (fewer DMAs = less ring overhead).
            XQ = KG * SH // 4
            for q in range(4):
                nc.sync.dma_start(
                    out=xgs[0][:, q * XQ : (q + 1) * XQ],
                    in_=xt.ap()[0][:, q * XQ : (q + 1) * XQ],
                )
            for h in range(2):
                nc.sync.dma_start(
                    out=xgs[1][:, h * 2 * XQ : (h + 1) * 2 * XQ],
                    in_=xt.ap()[1][:, h * 2 * XQ : (h + 1) * 2 * XQ],
                )
            for g in range(2, XG):
                nc.sync.dma_start(out=xgs[g][:], in_=xt.ap()[g])
            for h in range(2):
                load_w_half(0, h)
                load_w_half(1, h)

            # bt4/low4: [SCALE*B ; bias] and [low ; ones] replicated at
            # partition row-blocks 0/32/64/96 for the row-tiled stop packs
            # (bt4 comes pre-replicated from the host as one DMA).
            bt4 = cp.tile([P, D_OUT], bf16, name="bt4")
            nc.scalar.dma_start(out=bt4[:], in_=bga.ap())
            load_w_strip(2)
            load_w_strip(3)
            low4 = cp.tile([P, SH], bf16, name="low4")
            nc.gpsimd.memset(low4[:], 1.0)  # rows 16/48/80/112 = the ones rows

            def xsl(k, t):
                g, kg = divmod(k, KG)
                return xgs[g][:, kg * SH + t * TN : kg * SH + (t + 1) * TN]

            # Low psums: slot j at column-group j computes the t=(j%2) half;
            # each lands at partition block 32j for an aligned SBUF copy.
            pls = [pp.tile([P, TN], f32, name="ps") for _ in range(4)]

            def low_pack(k):
                for j in range(4):
                    nc.tensor.matmul(
                        pls[j][32 * j : 32 * j + R, :],
                        at[:, k * R : (k + 1) * R],
                        xsl(k, j % 2),
                        start=(k == 0),
                        stop=(k == KC - 1),
                        tile_position=(0, 32 * j),
                    )

            def stop_mm(ps, o, t, slot):
                nc.tensor.matmul(
                    ps[:],
                    bt4[32 * slot : 32 * slot + RA, o * P : (o + 1) * P],
                    low4[32 * slot : 32 * slot + RA, t * TN : (t + 1) * TN],
                    start=False,
                    stop=True,
                    tile_position=(32 * slot, 0),
                )

            # Pair 0 (strips 0/1): k-loop interleaved with the x arrivals.
            ps0 = [
                [pp.tile([P, TN], f32, name="ps") for _ in range(TGROUPS)]
                for _ in range(N_START)
            ]
            for g in range(XG):
                for k in range(g * KG, (g + 1) * KG):
                    low_pack(k)
                for s in range(N_START):
                    for k in range(g * KG, (g + 1) * KG):
                        for t in range(TGROUPS):
                            nc.tensor.matmul(
                                ps0[s][t][:],
                                w_strips[s][:, k * P : (k + 1) * P],
                                xsl(k, t),
                                start=(k == 0),
                                stop=False,
                            )
            for j in range(4):
                nc.vector.tensor_copy(
                    low4[32 * j : 32 * j + R, (j % 2) * TN : (j % 2 + 1) * TN],
                    pls[j][32 * j : 32 * j + R, :],
                )

            def strip_out(o, ps_pair):
                otile = op.tile([P, SH], bf16, name="otile")
                for t in range(TGROUPS):
                    nc.vector.tensor_copy(
                        otile[:, t * TN : (t + 1) * TN], ps_pair[t][:]
                    )
                nc.sync.dma_start(
                    out=ot_d.ap()[o * P : (o + 1) * P, :], in_=otile[:]
                )

            def finish_pair(o, ps_quad):
                for s in range(2):
                    for t in range(TGROUPS):
                        stop_mm(ps_quad[s][t], o + s, t, 2 * s + t)
                for s in range(2):
                    strip_out(o + s, ps_quad[s])

            finish_pair(0, ps0)

            def k_loop(o, t):
                ps = pp.tile([P, TN], f32, name="ps")
                wk = w_strips[o]
                for k in range(KC):
                    nc.tensor.matmul(
                        ps[:],
                        wk[:, k * P : (k + 1) * P],
                        xsl(k, t),
                        start=(k == 0),
                        stop=False,
                    )
                return ps

            # Steady pairs (2,3) .. (28,29).
            for o in range(N_START, O_CHUNKS - 2, 2):
                for j in (o + 2, o + 3):
                    if j < O_CHUNKS and j not in w_strips:
                        load_w_strip(j)
                ps_quad = [[k_loop(o + s, t) for t in range(TGROUPS)] for s in range(2)]
                w_strips.pop(o)
                w_strips.pop(o + 1)
                finish_pair(o, ps_quad)

            # Strip 30: 2-packed stop.
            o = O_CHUNKS - 2
            ps_pair = [k_loop(o, t) for t in range(TGROUPS)]
            for t in range(TGROUPS):
                stop_mm(ps_pair[t], o, t, t)
            strip_out(o, ps_pair)
            w_strips.pop(o)

            # Strip 31: per-half finish so the t=0 store overlaps the t=1
            # k-loop (shorter kernel tail).
            o = O_CHUNKS - 1
            otile = op.tile([P, SH], bf16, name="otile")
            for t in range(TGROUPS):
                ps = k_loop(o, t)
                stop_mm(ps, o, t, t)
                nc.vector.tensor_copy(otile[:, t * TN : (t + 1) * TN], ps[:])
                nc.sync.dma_start(
                    out=ot_d.ap()[o * P : (o + 1) * P, t * TN : (t + 1) * TN],
                    in_=otile[:, t * TN : (t + 1) * TN],
                )
    nc.compile()
    return nc


def _get_nc():
    global _cached_nc
    if _cached_nc is None:
        _cached_nc = _build()
    return _cached_nc


def _in_maps(x, weight, bias, lora_A, lora_B):
    # W^T packed as [o_chunk, partition, k*128+c]: element (o*128+c, k*128+p)
    # of W -> wt[o, p, k*128+c]; shared by all cores.
    wt = np.ascontiguousarray(
        weight.T.reshape(KC, P, O_CHUNKS, P).transpose(2, 1, 0, 3).reshape(
            O_CHUNKS, P, KC * P
        )
    ).astype(BF16)
    bias = bias.astype(np.float32, copy=False)
    maps = []
    for c in range(8):
        b, h = divmod(c, 2)
        xtc = np.ascontiguousarray(
            x[b, h * SH : (h + 1) * SH, :].T.reshape(XG, KG, P, SH)
            .transpose(0, 2, 1, 3)
            .reshape(XG, P, KG * SH)
        ).astype(BF16)
        apk = np.ascontiguousarray(
            lora_A[b].reshape(KC, P, R).transpose(1, 0, 2).reshape(P, KC * R)
        ).astype(BF16)
        baug1 = np.concatenate(
            [lora_B[b].astype(np.float32) * np.float32(SCALE), bias[None, :]], axis=0
        ).astype(BF16)
        baug = np.zeros((P, D_OUT), BF16)
        for i in range(4):
            baug[32 * i : 32 * i + RA] = baug1
        maps.append({"xt": xtc, "wt": wt, "apack": apk, "baug": baug})
    return maps


def kernel(x, weight, bias, lora_A, lora_B, _trace=False, _tmpdir=None):
    x = np.asarray(x, dtype=np.float32)
    weight = np.asarray(weight, dtype=np.float32)
    bias = np.asarray(bias, dtype=np.float32)
    lora_A = np.asarray(lora_A, dtype=np.float32)
    lora_B = np.asarray(lora_B, dtype=np.float32)

    nc = _get_nc()
    maps = _in_maps(x, weight, bias, lora_A, lora_B)
    res = run_bass_kernel_spmd(
        nc, maps, list(range(8)), trace=_trace, tmpdir=_tmpdir
    )
    out = np.empty((B, S, D_OUT), np.float32)
    for c in range(8):
        b, h = divmod(c, 2)
        out[b, h * SH : (h + 1) * SH, :] = res.results[c]["ot"].T.astype(np.float32)
    if _trace:
        return out, res
    return out


# revision 15
# speedup vs baseline: 1.0134x; 1.0134x over previous
"""HNLoRALinear Trainium2 kernel (bf16, packed-LoRA edition).

out[b,s,o] = x[b] @ W^T + bias + SCALE * (x[b] @ A[b]) @ B[b]

Sharding: 8 cores = 4 batches x 2 sequence-halves. Each core computes
its [1024 tokens, 4096 outs] output block, TRANSPOSED on device
(outs on PSUM partitions, tokens as the moving dim):
  - stationary operand: [128, 128] W^T chunk (bf16 -> fast weight load,
    fully hidden under the 512-column moving stream),
  - moving operand: 512-token slice of the SBUF-resident x^T (bf16),
  - consecutive k-chunk matmuls accumulate in fp32 PSUM.

All device data is bf16 (inputs rounded on host, output written bf16
and upcast on host): halves DMA traffic vs f32, enables the FWL
weight-load path, ~2.5e-3 rel err (gate is 2e-2).

Startup: ~10 dummy warm-up matmuls (HAM un-throttle) while the first
tiles stream in; all input DMAs ride ONE HWDGE ring (nc.sync) in
need-order (at, xg0, w0, xg1, w1, xg2, ..., xg15, w2, w3, bt) -- with
two rings the SDMA engines drain whole per-engine shares of one ring
before returning to the other, which stalled xg0's completion ~10us.
The first two o-strips' partial-k matmuls interleave with the x tile
arrivals so the PE has work for the whole x-load window.

LoRA work is packed into PE 32x32 sub-tiles:
  - low = (x @ A)^T: 4-way COLUMN tiling -- slots j=0..3 at
    tile_position (0, 32j) compute (t0, t1, t0, t1) concurrently
    (t-halves duplicated so each 32-row block of `low4` gets its values
    via a partition-ALIGNED PSUM->SBUF copy; no cross-partition moves).
  - the bias+lora stop matmul (K=17: [SCALE*B ; bias] @ [low ; ones]):
    4-way ROW tiling across a pair of o-strips -- slots i=2s+t at
    tile_position (32i, 0) read bt4/low4 rows 32i..32i+16.
This turns 128 full-cost matmuls into ~48 packed spans.
"""
import numpy as np
import ml_dtypes

import concourse.bass as bass  # noqa: F401  (bass must import before tile)
import concourse.mybir as mybir
import concourse.tile as tile
from concourse import bacc
from concourse.bass_utils import run_bass_kernel_spmd

# Problem shapes (hardcoded per contract).
B, S, D_IN, D_OUT, R = 4, 2048, 4096, 4096, 16
XG = 4                 # x DMA groups (separate tiles so deps are per-group)
SCALE = 32.0 / 16.0
SH = S // 2            # tokens per core
P = 128
KC = D_IN // P         # 32 contraction chunks
O_CHUNKS = D_OUT // P  # 32 output-feature chunks (PSUM partition dim)
TN = 512               # moving-dim token group width
TGROUPS = SH // TN     # 2
KG = KC // XG          # k-chunks per x group
RA = R + 1             # augmented rank (lora + bias row)
N_START = 2            # o-strips whose partial-k matmuls interleave with x load
N_WARM = 30            # dummy warm-up matmuls; sized to bridge until xg0 lands

BF16 = ml_dtypes.bfloat16

_cached_nc = None


def _build():
    bf16 = mybir.dt.bfloat16
    f32 = mybir.dt.float32
    nc = bacc.Bacc(
        "TRN2", target_bir_lowering=False, debug=False, enable_asserts=False
    )
    xt = nc.dram_tensor("xt", [XG, P, KG * SH], bf16, kind="ExternalInput")
    wt = nc.dram_tensor("wt", [O_CHUNKS, P, KC * P], bf16, kind="ExternalInput")
    apk = nc.dram_tensor("apack", [P, KC * R], bf16, kind="ExternalInput")
    bga = nc.dram_tensor("baug", [P, D_OUT], bf16, kind="ExternalInput")
    ot_d = nc.dram_tensor("ot", [D_OUT, SH], bf16, kind="ExternalOutput")

    with tile.TileContext(nc) as tc:
        with (
            tc.tile_pool(name="xp", bufs=1) as xp,
            tc.tile_pool(name="wp", bufs=5) as wp,
            tc.tile_pool(name="cp", bufs=1) as cp,
            tc.tile_pool(name="op", bufs=3) as op,
            tc.tile_pool(name="pp", bufs=8, space="PSUM") as pp,
        ):
            # HAM warm-up: the PE clock sits at 1.2 GHz until it has seen
            # ~3.4us of sustained matmul activity. Burn that window on dummy
            # matmuls over a memset tile (no DMA dependency) while the first
            # x/W tiles stream in, so the real matmuls run at 2.4 GHz.
            wu_sb = cp.tile([P, TN], bf16, name="wusb")
            nc.vector.memset(wu_sb[:], 0.0)
            wu_ps = pp.tile([P, TN], f32, name="ps")
            for i in range(N_WARM):
                nc.tensor.matmul(
                    wu_ps[:],
                    wu_sb[:, 0:P],
                    wu_sb[:],
                    start=(i == 0),
                    stop=(i == N_WARM - 1),
                )

            # Single-ring input DMA schedule, in PE-need order.
            at = cp.tile([P, KC * R], bf16, name="at")
            nc.sync.dma_start(out=at[:], in_=apk.ap())

            xgs = [
                xp.tile([P, KG * SH], bf16, name=f"xg{g}", tag=f"xg{g}")
                for g in range(XG)
            ]
            w_strips = {}

            def load_w_strip(o):
                # One fully-contiguous 2D DMA per strip (host pre-packs W
                # as [o_chunk, partition, k*128+c]).
                wk = wp.tile([P, KC * P], bf16, name="wk")
                nc.scalar.dma_start(out=wk[:], in_=wt.ap()[o])
                w_strips[o] = wk

            # Strips 0/1 stream in k-halves: with KG=8, half h of a strip
            # covers k-groups the startup block reaches after x-group 2h,
            # so it never waits on a whole-strip load.
            HW_ = KC * P // 2
            for o in range(N_START):
                w_strips[o] = wp.tile([P, KC * P], bf16, name="wk")

            def load_w_half(o, h):
                nc.scalar.dma_start(
                    out=w_strips[o][:, h * HW_ : (h + 1) * HW_],
                    in_=wt.ap()[o][:, h * HW_ : (h + 1) * HW_],
                )

            # Two HWDGE rings in parallel: x (+ outputs) on the SP ring,
            # all W/bias traffic on the Activation ring. Few, big DMAs:
            # per-DMA ring/completion overhead (~1.5us) is what starves
            # the startup, not bytes.
            for g in range(XG):
                nc.sync.dma_start(out=xgs[g][:], in_=xt.ap()[g])
            for h in range(2):
                load_w_half(0, h)
                load_w_half(1, h)

            # bt4/low4: [SCALE*B ; bias] and [low ; ones] replicated at
            # partition row-blocks 0/32/64/96 for the row-tiled stop packs
            # (bt4 comes pre-replicated from the host as one DMA).
            bt4 = cp.tile([P, D_OUT], bf16, name="bt4")
            nc.scalar.dma_start(out=bt4[:], in_=bga.ap())
            load_w_strip(2)
            load_w_strip(3)
            low4 = cp.tile([P, SH], bf16, name="low4")
            nc.gpsimd.memset(low4[:], 1.0)  # rows 16/48/80/112 = the ones rows

            def xsl(k, t):
                g, kg = divmod(k, KG)
                return xgs[g][:, kg * SH + t * TN : kg * SH + (t + 1) * TN]

            # Low psums: slot j at column-group j computes the t=(j%2) half;
            # each lands at partition block 32j for an aligned SBUF copy.
            pls = [pp.tile([P, TN], f32, name="ps") for _ in range(4)]

            def low_pack(k):
                for j in range(4):
                    nc.tensor.matmul(
                        pls[j][32 * j : 32 * j + R, :],
                        at[:, k * R : (k + 1) * R],
                        xsl(k, j % 2),
                        start=(k == 0),
                        stop=(k == KC - 1),
                        tile_position=(0, 32 * j),
                    )

            def stop_mm(ps, o, t, slot):
                nc.tensor.matmul(
                    ps[:],
                    bt4[32 * slot : 32 * slot + RA, o * P : (o + 1) * P],
                    low4[32 * slot : 32 * slot + RA, t * TN : (t + 1) * TN],
                    start=False,
                    stop=True,
                    tile_position=(32 * slot, 0),
                )

            # Pair 0 (strips 0/1): k-loop interleaved with the x arrivals.
            ps0 = [
                [pp.tile([P, TN], f32, name="ps") for _ in range(TGROUPS)]
                for _ in range(N_START)
            ]
            for g in range(XG):
                for k in range(g * KG, (g + 1) * KG):
                    low_pack(k)
                for s in range(N_START):
                    for k in range(g * KG, (g + 1) * KG):
                        for t in range(TGROUPS):
                            nc.tensor.matmul(
                                ps0[s][t][:],
                                w_strips[s][:, k * P : (k + 1) * P],
                                xsl(k, t),
                                start=(k == 0),
                                stop=False,
                            )
            for j in range(4):
                nc.vector.tensor_copy(
                    low4[32 * j : 32 * j + R, (j % 2) * TN : (j % 2 + 1) * TN],
                    pls[j][32 * j : 32 * j + R, :],
                )

            def strip_out(o, ps_pair):
                otile = op.tile([P, SH], bf16, name="otile")
                for t in range(TGROUPS):
                    nc.vector.tensor_copy(
                        otile[:, t * TN : (t + 1) * TN], ps_pair[t][:]
                    )
                nc.sync.dma_start(
                    out=ot_d.ap()[o * P : (o + 1) * P, :], in_=otile[:]
                )

            def finish_pair(o, ps_quad):
                for s in range(2):
                    for t in range(TGROUPS):
                        stop_mm(ps_quad[s][t], o + s, t, 2 * s + t)
                for s in range(2):
                    strip_out(o + s, ps_quad[s])

            finish_pair(0, ps0)

            def k_loop(o, t):
                ps = pp.tile([P, TN], f32, name="ps")
                wk = w_strips[o]
                for k in range(KC):
                    nc.tensor.matmul(
                        ps[:],
                        wk[:, k * P : (k + 1) * P],
                        xsl(k, t),
                        start=(k == 0),
                        stop=False,
                    )
                return ps

            # Steady pairs (2,3) .. (28,29).
            for o in range(N_START, O_CHUNKS - 2, 2):
                for j in (o + 2, o + 3):
                    if j < O_CHUNKS and j not in w_strips:
                        load_w_strip(j)
                ps_quad = [[k_loop(o + s, t) for t in range(TGROUPS)] for s in range(2)]
                w_strips.pop(o)
                w_strips.pop(o + 1)
                finish_pair(o, ps_quad)

            # Strip 30: 2-packed stop.
            o = O_CHUNKS - 2
            ps_pair = [k_loop(o, t) for t in range(TGROUPS)]
            for t in range(TGROUPS):
                stop_mm(ps_pair[t], o, t, t)
            strip_out(o, ps_pair)
            w_strips.pop(o)

            # Strip 31: per-half finish so the t=0 store overlaps the t=1
            # k-loop (shorter kernel tail).
            o = O_CHUNKS - 1
            otile = op.tile([P, SH], bf16, name="otile")
            for t in range(TGROUPS):
                ps = k_loop(o, t)
                stop_mm(ps, o, t, t)
                nc.vector.tensor_copy(otile[:, t * TN : (t + 1) * TN], ps[:])
                nc.sync.dma_start(
                    out=ot_d.ap()[o * P : (o + 1) * P, t * TN : (t + 1) * TN],
                    in_=otile[:, t * TN : (t + 1) * TN],
                )
    nc.compile()
    return nc


def _get_nc():
    global _cached_nc
    if _cached_nc is None:
        _cached_nc = _build()
    return _cached_nc


def _in_maps(x, weight, bias, lora_A, lora_B):
    # W^T packed as [o_chunk, partition, k*128+c]: element (o*128+c, k*128+p)
    # of W -> wt[o, p, k*128+c]; shared by all cores.
    wt = np.ascontiguousarray(
        weight.T.reshape(KC, P, O_CHUNKS, P).transpose(2, 1, 0, 3).reshape(
            O_CHUNKS, P, KC * P
        )
    ).astype(BF16)
    bias = bias.astype(np.float32, copy=False)
    maps = []
    for c in range(8):
        b, h = divmod(c, 2)
        xtc = np.ascontiguousarray(
            x[b, h * SH : (h + 1) * SH, :].T.reshape(XG, KG, P, SH)
            .transpose(0, 2, 1, 3)
            .reshape(XG, P, KG * SH)
        ).astype(BF16)
        apk = np.ascontiguousarray(
            lora_A[b].reshape(KC, P, R).transpose(1, 0, 2).reshape(P, KC * R)
        ).astype(BF16)
        baug1 = np.concatenate(
            [lora_B[b].astype(np.float32) * np.float32(SCALE), bias[None, :]], axis=0
        ).astype(BF16)
        baug = np.zeros((P, D_OUT), BF16)
        for i in range(4):
            baug[32 * i : 32 * i + RA] = baug1
        maps.append({"xt": xtc, "wt": wt, "apack": apk, "baug": baug})
    return maps


def kernel(x, weight, bias, lora_A, lora_B, _trace=False, _tmpdir=None):
    x = np.asarray(x, dtype=np.float32)
    weight = np.asarray(weight, dtype=np.float32)
    bias = np.asarray(bias, dtype=np.float32)
    lora_A = np.asarray(lora_A, dtype=np.float32)
    lora_B = np.asarray(lora_B, dtype=np.float32)

    nc = _get_nc()
    maps = _in_maps(x, weight, bias, lora_A, lora_B)
    res = run_bass_kernel_spmd(
        nc, maps, list(range(8)), trace=_trace, tmpdir=_tmpdir
    )
    out = np.empty((B, S, D_OUT), np.float32)
    for c in range(8):
        b, h = divmod(c, 2)
        out[b, h * SH : (h + 1) * SH, :] = res.results[c]["ot"].T.astype(np.float32)
    if _trace:
        return out, res
    return out


# revision 16
# speedup vs baseline: 1.0192x; 1.0057x over previous
"""HNLoRALinear Trainium2 kernel (bf16, packed-LoRA edition).

out[b,s,o] = x[b] @ W^T + bias + SCALE * (x[b] @ A[b]) @ B[b]

Sharding: 8 cores = 4 batches x 2 sequence-halves. Each core computes
its [1024 tokens, 4096 outs] output block, TRANSPOSED on device
(outs on PSUM partitions, tokens as the moving dim):
  - stationary operand: [128, 128] W^T chunk (bf16 -> fast weight load,
    fully hidden under the 512-column moving stream),
  - moving operand: 512-token slice of the SBUF-resident x^T (bf16),
  - consecutive k-chunk matmuls accumulate in fp32 PSUM.

All device data is bf16 (inputs rounded on host, output written bf16
and upcast on host): halves DMA traffic vs f32, enables the FWL
weight-load path, ~2.5e-3 rel err (gate is 2e-2).

Startup: ~10 dummy warm-up matmuls (HAM un-throttle) while the first
tiles stream in; all input DMAs ride ONE HWDGE ring (nc.sync) in
need-order (at, xg0, w0, xg1, w1, xg2, ..., xg15, w2, w3, bt) -- with
two rings the SDMA engines drain whole per-engine shares of one ring
before returning to the other, which stalled xg0's completion ~10us.
The first two o-strips' partial-k matmuls interleave with the x tile
arrivals so the PE has work for the whole x-load window.

LoRA work is packed into PE 32x32 sub-tiles:
  - low = (x @ A)^T: 4-way COLUMN tiling -- slots j=0..3 at
    tile_position (0, 32j) compute (t0, t1, t0, t1) concurrently
    (t-halves duplicated so each 32-row block of `low4` gets its values
    via a partition-ALIGNED PSUM->SBUF copy; no cross-partition moves).
  - the bias+lora stop matmul (K=17: [SCALE*B ; bias] @ [low ; ones]):
    4-way ROW tiling across a pair of o-strips -- slots i=2s+t at
    tile_position (32i, 0) read bt4/low4 rows 32i..32i+16.
This turns 128 full-cost matmuls into ~48 packed spans.
"""
import numpy as np
import ml_dtypes

import concourse.bass as bass  # noqa: F401  (bass must import before tile)
import concourse.mybir as mybir
import concourse.tile as tile
from concourse import bacc
from concourse.bass_utils import run_bass_kernel_spmd

# Problem shapes (hardcoded per contract).
B, S, D_IN, D_OUT, R = 4, 2048, 4096, 4096, 16
XG = 4                 # x DMA groups (separate tiles so deps are per-group)
SCALE = 32.0 / 16.0
SH = S // 2            # tokens per core
P = 128
KC = D_IN // P         # 32 contraction chunks
O_CHUNKS = D_OUT // P  # 32 output-feature chunks (PSUM partition dim)
TN = 512               # moving-dim token group width
TGROUPS = SH // TN     # 2
KG = KC // XG          # k-chunks per x group
RA = R + 1             # augmented rank (lora + bias row)
N_START = 2            # o-strips whose partial-k matmuls interleave with x load
N_WARM = 14            # dummy warm-up matmuls; bridge until xg0's first half lands

BF16 = ml_dtypes.bfloat16

_cached_nc = None


def _build():
    bf16 = mybir.dt.bfloat16
    f32 = mybir.dt.float32
    nc = bacc.Bacc(
        "TRN2", target_bir_lowering=False, debug=False, enable_asserts=False
    )
    xt = nc.dram_tensor("xt", [XG, P, KG * SH], bf16, kind="ExternalInput")
    wt = nc.dram_tensor("wt", [O_CHUNKS, P, KC * P], bf16, kind="ExternalInput")
    apk = nc.dram_tensor("apack", [P, KC * R], bf16, kind="ExternalInput")
    bga = nc.dram_tensor("baug", [P, D_OUT], bf16, kind="ExternalInput")
    ot_d = nc.dram_tensor("ot", [D_OUT, SH], bf16, kind="ExternalOutput")

    with tile.TileContext(nc) as tc:
        with (
            tc.tile_pool(name="xp", bufs=1) as xp,
            tc.tile_pool(name="wp", bufs=5) as wp,
            tc.tile_pool(name="cp", bufs=1) as cp,
            tc.tile_pool(name="op", bufs=3) as op,
            tc.tile_pool(name="pp", bufs=8, space="PSUM") as pp,
        ):
            # HAM warm-up: the PE clock sits at 1.2 GHz until it has seen
            # ~3.4us of sustained matmul activity. Burn that window on dummy
            # matmuls over a memset tile (no DMA dependency) while the first
            # x/W tiles stream in, so the real matmuls run at 2.4 GHz.
            wu_sb = cp.tile([P, TN], bf16, name="wusb")
            nc.vector.memset(wu_sb[:], 0.0)
            wu_ps = pp.tile([P, TN], f32, name="ps")
            for i in range(N_WARM):
                nc.tensor.matmul(
                    wu_ps[:],
                    wu_sb[:, 0:P],
                    wu_sb[:],
                    start=(i == 0),
                    stop=(i == N_WARM - 1),
                )

            # Single-ring input DMA schedule, in PE-need order.
            at = cp.tile([P, KC * R], bf16, name="at")
            nc.sync.dma_start(out=at[:], in_=apk.ap())

            xgs = [
                xp.tile([P, KG * SH], bf16, name=f"xg{g}", tag=f"xg{g}")
                for g in range(XG)
            ]
            w_strips = {}

            def load_w_strip(o):
                # One fully-contiguous 2D DMA per strip (host pre-packs W
                # as [o_chunk, partition, k*128+c]).
                wk = wp.tile([P, KC * P], bf16, name="wk")
                nc.scalar.dma_start(out=wk[:], in_=wt.ap()[o])
                w_strips[o] = wk

            # Strips 0/1 stream in k-halves: with KG=8, half h of a strip
            # covers k-groups the startup block reaches after x-group 2h,
            # so it never waits on a whole-strip load.
            HW_ = KC * P // 2
            for o in range(N_START):
                w_strips[o] = wp.tile([P, KC * P], bf16, name="wk")

            def load_w_half(o, h):
                nc.scalar.dma_start(
                    out=w_strips[o][:, h * HW_ : (h + 1) * HW_],
                    in_=wt.ap()[o][:, h * HW_ : (h + 1) * HW_],
                )

            # Two HWDGE rings in parallel: x (+ outputs) on the SP ring,
            # all W/bias traffic on the Activation ring. Few, big DMAs:
            # per-DMA ring/completion overhead (~1.5us) is what starves
            # the startup, not bytes.
            # xg0 alone lands as two 1MB halves: the first piece arrives
            # ~4us sooner than a whole 2MB tile, bridging the end of the
            # warm-up into real work with HAM still hot.
            XH = KG * SH // 2
            for h in range(2):
                nc.sync.dma_start(
                    out=xgs[0][:, h * XH : (h + 1) * XH],
                    in_=xt.ap()[0][:, h * XH : (h + 1) * XH],
                )
            for g in range(1, XG):
                nc.sync.dma_start(out=xgs[g][:], in_=xt.ap()[g])
            for h in range(2):
                load_w_half(0, h)
                load_w_half(1, h)

            # bt4/low4: [SCALE*B ; bias] and [low ; ones] replicated at
            # partition row-blocks 0/32/64/96 for the row-tiled stop packs
            # (bt4 comes pre-replicated from the host as one DMA).
            bt4 = cp.tile([P, D_OUT], bf16, name="bt4")
            nc.scalar.dma_start(out=bt4[:], in_=bga.ap())
            load_w_strip(2)
            load_w_strip(3)
            low4 = cp.tile([P, SH], bf16, name="low4")
            nc.gpsimd.memset(low4[:], 1.0)  # rows 16/48/80/112 = the ones rows

            def xsl(k, t):
                g, kg = divmod(k, KG)
                return xgs[g][:, kg * SH + t * TN : kg * SH + (t + 1) * TN]

            # Low psums: slot j at column-group j computes the t=(j%2) half;
            # each lands at partition block 32j for an aligned SBUF copy.
            pls = [pp.tile([P, TN], f32, name="ps") for _ in range(4)]

            def low_pack(k):
                for j in range(4):
                    nc.tensor.matmul(
                        pls[j][32 * j : 32 * j + R, :],
                        at[:, k * R : (k + 1) * R],
                        xsl(k, j % 2),
                        start=(k == 0),
                        stop=(k == KC - 1),
                        tile_position=(0, 32 * j),
                    )

            def stop_mm(ps, o, t, slot):
                nc.tensor.matmul(
                    ps[:],
                    bt4[32 * slot : 32 * slot + RA, o * P : (o + 1) * P],
                    low4[32 * slot : 32 * slot + RA, t * TN : (t + 1) * TN],
                    start=False,
                    stop=True,
                    tile_position=(32 * slot, 0),
                )

            # Pair 0 (strips 0/1): k-loop interleaved with the x arrivals.
            ps0 = [
                [pp.tile([P, TN], f32, name="ps") for _ in range(TGROUPS)]
                for _ in range(N_START)
            ]
            for g in range(XG):
                for k in range(g * KG, (g + 1) * KG):
                    low_pack(k)
                for s in range(N_START):
                    for k in range(g * KG, (g + 1) * KG):
                        for t in range(TGROUPS):
                            nc.tensor.matmul(
                                ps0[s][t][:],
                                w_strips[s][:, k * P : (k + 1) * P],
                                xsl(k, t),
                                start=(k == 0),
                                stop=False,
                            )
            for j in range(4):
                nc.vector.tensor_copy(
                    low4[32 * j : 32 * j + R, (j % 2) * TN : (j % 2 + 1) * TN],
                    pls[j][32 * j : 32 * j + R, :],
                )

            def strip_out(o, ps_pair):
                otile = op.tile([P, SH], bf16, name="otile")
                for t in range(TGROUPS):
                    nc.vector.tensor_copy(
                        otile[:, t * TN : (t + 1) * TN], ps_pair[t][:]
                    )
                nc.sync.dma_start(
                    out=ot_d.ap()[o * P : (o + 1) * P, :], in_=otile[:]
                )

            def finish_pair(o, ps_quad):
                for s in range(2):
                    for t in range(TGROUPS):
                        stop_mm(ps_quad[s][t], o + s, t, 2 * s + t)
                for s in range(2):
                    strip_out(o + s, ps_quad[s])

            finish_pair(0, ps0)

            def k_loop(o, t):
                ps = pp.tile([P, TN], f32, name="ps")
                wk = w_strips[o]
                for k in range(KC):
                    nc.tensor.matmul(
                        ps[:],
                        wk[:, k * P : (k + 1) * P],
                        xsl(k, t),
                        start=(k == 0),
                        stop=False,
                    )
                return ps

            # Steady pairs (2,3) .. (28,29).
            for o in range(N_START, O_CHUNKS - 2, 2):
                for j in (o + 2, o + 3):
                    if j < O_CHUNKS and j not in w_strips:
                        load_w_strip(j)
                ps_quad = [[k_loop(o + s, t) for t in range(TGROUPS)] for s in range(2)]
                w_strips.pop(o)
                w_strips.pop(o + 1)
                finish_pair(o, ps_quad)

            # Strip 30: 2-packed stop.
            o = O_CHUNKS - 2
            ps_pair = [k_loop(o, t) for t in range(TGROUPS)]
            for t in range(TGROUPS):
                stop_mm(ps_pair[t], o, t, t)
            strip_out(o, ps_pair)
            w_strips.pop(o)

            # Strip 31: per-half finish so the t=0 store overlaps the t=1
            # k-loop (shorter kernel tail).
            o = O_CHUNKS - 1
            otile = op.tile([P, SH], bf16, name="otile")
            for t in range(TGROUPS):
                ps = k_loop(o, t)
                stop_mm(ps, o, t, t)
                nc.vector.tensor_copy(otile[:, t * TN : (t + 1) * TN], ps[:])
                nc.sync.dma_start(
                    out=ot_d.ap()[o * P : (o + 1) * P, t * TN : (t + 1) * TN],
                    in_=otile[:, t * TN : (t + 1) * TN],
                )
    nc.compile()
    return nc


def _get_nc():
    global _cached_nc
    if _cached_nc is None:
        _cached_nc = _build()
    return _cached_nc


def _in_maps(x, weight, bias, lora_A, lora_B):
    # W^T packed as [o_chunk, partition, k*128+c]: element (o*128+c, k*128+p)
    # of W -> wt[o, p, k*128+c]; shared by all cores.
    wt = np.ascontiguousarray(
        weight.T.reshape(KC, P, O_CHUNKS, P).transpose(2, 1, 0, 3).reshape(
            O_CHUNKS, P, KC * P
        )
    ).astype(BF16)
    bias = bias.astype(np.float32, copy=False)
    maps = []
    for c in range(8):
        b, h = divmod(c, 2)
        xtc = np.ascontiguousarray(
            x[b, h * SH : (h + 1) * SH, :].T.reshape(XG, KG, P, SH)
            .transpose(0, 2, 1, 3)
            .reshape(XG, P, KG * SH)
        ).astype(BF16)
        apk = np.ascontiguousarray(
            lora_A[b].reshape(KC, P, R).transpose(1, 0, 2).reshape(P, KC * R)
        ).astype(BF16)
        baug1 = np.concatenate(
            [lora_B[b].astype(np.float32) * np.float32(SCALE), bias[None, :]], axis=0
        ).astype(BF16)
        baug = np.zeros((P, D_OUT), BF16)
        for i in range(4):
            baug[32 * i : 32 * i + RA] = baug1
        maps.append({"xt": xtc, "wt": wt, "apack": apk, "baug": baug})
    return maps


def kernel(x, weight, bias, lora_A, lora_B, _trace=False, _tmpdir=None):
    x = np.asarray(x, dtype=np.float32)
    weight = np.asarray(weight, dtype=np.float32)
    bias = np.asarray(bias, dtype=np.float32)
    lora_A = np.asarray(lora_A, dtype=np.float32)
    lora_B = np.asarray(lora_B, dtype=np.float32)

    nc = _get_nc()
    maps = _in_maps(x, weight, bias, lora_A, lora_B)
    res = run_bass_kernel_spmd(
        nc, maps, list(range(8)), trace=_trace, tmpdir=_tmpdir
    )
    out = np.empty((B, S, D_OUT), np.float32)
    for c in range(8):
        b, h = divmod(c, 2)
        out[b, h * SH : (h + 1) * SH, :] = res.results[c]["ot"].T.astype(np.float32)
    if _trace:
        return out, res
    return out
